# revision 1
# baseline (speedup 1.0000x reference)
"""Trainium2 Bass kernel for nn_EventSampler (thinning / rejection sampling).

Contract: kernel(**inputs) takes the FULL unsharded inputs (as produced by
setup_inputs()) and returns the full output (res, weights), matching the
jax reference. Internally shards the batch dim (16) across 8 NeuronCores
(2 batches = 256 (b,l) pairs per core) and runs a single SPMD Bass program.

Per (b,l) pair (one SBUF partition per pair, 128 pairs per chunk, 2 chunks
per core):
  unified grid: ONE [26, M] softplus-sum evaluation per pair covering the 20
    bound-scan points dt_s = tds*s/19 AND 6 Chebyshev-Lobatto nodes on
    [0, D] (D >= xmax picked on host from a float64 bound estimate; only the
    interpolation domain, never the math, depends on it).
  bounds = 1.5 * max over the 20 scan values.
  tot(x) at the 256 scaled draws x_e = raw_e/bounds is evaluated as the
    degree-5 Chebyshev interpolant (Clenshaw); interpolation error sits at
    the f32 noise floor because tot is analytic on this tiny domain.
  accept[s,e] = unif[s,e]*bounds < tot_e ; accepted time = min accepted x_e,
    computed as bounds-free max of accept/raw_e then one reciprocal and a
    final *1/bounds; fallback = max(x_last, dtime_boundary).

softplus = Ln(exp(z)+1) on ScalarE (Exp and Ln share one ACT table set).
Stage-2's [S,E] elementwise ops are split between VectorE and GpSimd by
s-range; the VectorE instances read their per-e operand from PSUM so the two
engines never touch the shared SBUF port pair at the same time.
"""

import os
import sys

import numpy as np

for _p in ("/opt/trn_rl_repo",):
    if _p not in sys.path and os.path.isdir(_p):
        sys.path.insert(0, _p)

import concourse.bacc as bacc
import concourse.tile as tile
import concourse.mybir as mybir
from concourse.bass_utils import run_bass_kernel_spmd

F32 = mybir.dt.float32

# Problem constants (hardcoded per the harness contract).
B, L, M = 16, 128, 32
S, E, S0 = 32, 256, 20          # NUM_SAMPLE, NUM_EXP, NUM_BOUND
OVER = 1.5
KC = 5                          # Chebyshev-Lobatto nodes for tot(x)
G = S0 + KC                     # unified grid points per pair
GS = 24                         # s-rows of stage-2 handled by GpSimd
N_CORES = 8
BPC = B // N_CORES              # batches per core
P = BPC * L                     # (b,l) pairs per core = 256
NP = 128                        # SBUF partitions
NCHUNK = P // NP                # chunks per core = 2

_CACHE = {}


def _alu(name):
    return getattr(mybir.AluOpType, name)


def _act(name):
    return getattr(mybir.ActivationFunctionType, name)


def build_program(gs=GS):
    nc = bacc.Bacc("TRN2", target_bir_lowering=False, debug=False,
                   enable_asserts=False, num_devices=N_CORES)

    # Per-core DRAM I/O.
    u_d = nc.dram_tensor("u", [P, S, E], F32, kind="ExternalInput")
    raw_d = nc.dram_tensor("raw", [P, E], F32, kind="ExternalInput")
    tds_d = nc.dram_tensor("tds", [P, 1], F32, kind="ExternalInput")
    dtb_d = nc.dram_tensor("dtb", [P, 1], F32, kind="ExternalInput")
    aemb_d = nc.dram_tensor("aemb", [P, M], F32, kind="ExternalInput")
    nodes_d = nc.dram_tensor("nodes", [P, KC], F32, kind="ExternalInput")
    fourd_d = nc.dram_tensor("fourd", [P, 1], F32, kind="ExternalInput")
    # Host-replicated small constants ([NP, ...]).
    negbeta_d = nc.dram_tensor("negbeta", [NP, M], F32, kind="ExternalInput")
    mu_d = nc.dram_tensor("mu", [NP, M], F32, kind="ExternalInput")
    tlin_d = nc.dram_tensor("tlin", [NP, S0], F32, kind="ExternalInput")
    wfull_d = nc.dram_tensor("wfull", [NP, KC * KC], F32, kind="ExternalInput")
    res_d = nc.dram_tensor("res", [P, S], F32, kind="ExternalOutput")

    mult = _alu("mult")
    add = _alu("add")
    sub = _alu("subtract")
    is_lt = _alu("is_lt")
    is_gt = _alu("is_gt")
    amax = _alu("max")
    amin = _alu("min")
    Exp = _act("Exp")
    Cp = _act("Copy")
    Ln = _act("Ln")
    DS = S - gs                   # s-rows on DVE

    with tile.TileContext(nc) as tc:
        with (
            tc.tile_pool(name="const", bufs=1) as constp,
            tc.tile_pool(name="cps", bufs=1, space="PSUM") as cps,
            tc.tile_pool(name="pps", bufs=2, space="PSUM") as pps,
            tc.tile_pool(name="cbp", bufs=1, space="PSUM") as cbp,
            tc.tile_pool(name="ubuf", bufs=2) as ubuf,
            tc.tile_pool(name="slab", bufs=1) as slab,
            tc.tile_pool(name="small", bufs=2) as small,
            tc.tile_pool(name="clen", bufs=1) as clen,
        ):
            negbeta_t = constp.tile([NP, M], F32, tag="negbeta")
            nc.sync.dma_start(out=negbeta_t[:], in_=negbeta_d.ap())
            mu_t = constp.tile([NP, M], F32, tag="mu")
            nc.sync.dma_start(out=mu_t[:], in_=mu_d.ap())
            tlin_t = constp.tile([NP, S0], F32, tag="tlin")
            nc.sync.dma_start(out=tlin_t[:], in_=tlin_d.ap())
            wfull_t = constp.tile([NP, KC * KC], F32, tag="wfull")
            nc.sync.dma_start(out=wfull_t[:], in_=wfull_d.ap())
            # PSUM copies of per-m constants (second operands of DVE tt ops)
            consts_p = cps.tile([NP, 2 * M + KC * KC], F32, tag="consts_p")
            nc.vector.tensor_copy(consts_p[:, 0:M], negbeta_t[:])
            nc.vector.tensor_copy(consts_p[:, M:2 * M], mu_t[:])
            nc.vector.tensor_copy(consts_p[:, 2 * M:], wfull_t[:])
            nb_e = consts_p[:, 0:M].unsqueeze(1)           # [NP,1,M] PSUM
            mu_e = consts_p[:, M:2 * M].unsqueeze(1)       # [NP,1,M] PSUM
            wfull_p = consts_p[:, 2 * M:].rearrange("p (a b) -> p a b", a=KC)

            # ---- phase 0: all small DMAs for both chunks (issued before
            # the big u loads so both stage-0/1 chains can start immediately),
            # then the u slabs. ----
            ch = [dict() for _ in range(NCHUNK)]
            for c in range(NCHUNK):
                sl = slice(c * NP, (c + 1) * NP)
                d = ch[c]
                d["raw_t"] = small.tile([NP, E], F32, tag="raw", name=f"raw{c}")
                nc.sync.dma_start(out=d["raw_t"][:], in_=raw_d.ap()[sl])
                d["tds_t"] = small.tile([NP, 1], F32, tag="tds", name=f"tds{c}")
                nc.sync.dma_start(out=d["tds_t"][:], in_=tds_d.ap()[sl])
                d["dtb_t"] = small.tile([NP, 1], F32, tag="dtb", name=f"dtb{c}")
                nc.sync.dma_start(out=d["dtb_t"][:], in_=dtb_d.ap()[sl])
                d["fourd_t"] = small.tile([NP, 1], F32, tag="fourd", name=f"fourd{c}")
                nc.sync.dma_start(out=d["fourd_t"][:], in_=fourd_d.ap()[sl])
                d["aemb_t"] = small.tile([NP, M], F32, tag="aemb", name=f"aemb{c}")
                nc.sync.dma_start(out=d["aemb_t"][:], in_=aemb_d.ap()[sl])
                d["pts"] = small.tile([NP, G], F32, tag="pts", name=f"pts{c}")
                nc.sync.dma_start(out=d["pts"][:, S0:G], in_=nodes_d.ap()[sl])
            for c in range(NCHUNK):
                sl = slice(c * NP, (c + 1) * NP)
                ch[c]["u_t"] = ubuf.tile([NP, S, E], F32, tag="u", name=f"u{c}")
                nc.sync.dma_start(out=ch[c]["u_t"][:], in_=u_d.ap()[sl])

            # ---- phase 1: bounds + Chebyshev tot for both chunks ----
            for c in range(NCHUNK):
                d = ch[c]
                raw_t, tds_t, aemb_t, pts = d["raw_t"], d["tds_t"], d["aemb_t"], d["pts"]
                aemb_e = aemb_t[:].unsqueeze(1)
                nc.scalar.activation(pts[:, 0:S0], tlin_t[:], Cp, scale=tds_t[:])
                zG = small.tile([NP, G, M], F32, tag="gA")
                nc.vector.tensor_tensor(out=zG[:], in0=pts[:].unsqueeze(2).to_broadcast((NP, G, M)),
                                        in1=nb_e.to_broadcast((NP, G, M)), op=mult)
                dG = small.tile([NP, G, M], F32, tag="gB")
                nc.scalar.activation(dG[:], zG[:], Exp)
                gG = small.tile([NP, G, M], F32, tag="gA")
                nc.vector.tensor_tensor(out=gG[:], in0=dG[:],
                                        in1=aemb_e.to_broadcast((NP, G, M)), op=mult)
                sG = small.tile([NP, G, M], F32, tag="gB")
                nc.vector.tensor_tensor(out=sG[:], in0=gG[:],
                                        in1=mu_e.to_broadcast((NP, G, M)), op=add)
                eG = small.tile([NP, G, M], F32, tag="gA")
                nc.scalar.activation(eG[:], sG[:], Exp)
                spG = small.tile([NP, G, M], F32, tag="gB")
                nc.scalar.activation(spG[:], eG[:], Ln, bias=1.0)
                vals = small.tile([NP, G], F32, tag="vals")
                nc.vector.reduce_sum(out=vals[:], in_=spG[:], axis=mybir.AxisListType.X)

                bmax = small.tile([NP, 1], F32, tag="bmax")
                nc.vector.reduce_max(out=bmax[:], in_=vals[:, 0:S0],
                                     axis=mybir.AxisListType.X)
                b15 = small.tile([NP, 1], F32, tag="b15")
                nc.scalar.activation(b15[:], bmax[:], Cp, scale=float(OVER))
                invb = small.tile([NP, 1], F32, tag="invb")
                nc.vector.reciprocal(invb[:], b15[:])
                svc2 = small.tile([NP, 1], F32, tag="svc2")
                nc.scalar.activation(svc2[:], invb[:], Cp, scale=d["fourd_t"][:])
                w2 = small.tile([NP, E], F32, tag="w2")
                nc.scalar.activation(w2[:], raw_t[:], Cp, scale=svc2[:], bias=-2.0)
                v = small.tile([NP, E], F32, tag="v")
                nc.scalar.activation(v[:], w2[:], Cp, scale=0.5)
                rawrec = small.tile([NP, E], F32, tag="rawrec")
                nc.vector.reciprocal(rawrec[:], raw_t[:])
                pchunk = pps.tile([NP, 2 * E], F32, tag="pchunk")
                rawrec_p = pchunk[:, E:2 * E]
                nc.scalar.activation(rawrec_p, rawrec[:], Cp)

                cw = small.tile([NP, KC, KC], F32, tag="cw")
                nc.vector.tensor_tensor(out=cw[:], in0=vals[:, S0:G].unsqueeze(1).to_broadcast((NP, KC, KC)),
                                        in1=wfull_p, op=mult)
                cc = small.tile([NP, KC], F32, tag="cc")
                nc.vector.reduce_sum(out=cc[:], in_=cw[:], axis=mybir.AxisListType.X)

                b1 = cbp.tile([NP, E], F32, tag="cbi")
                nc.vector.tensor_scalar(out=b1[:], in0=w2[:], scalar1=cc[:, KC - 1:KC],
                                        scalar2=cc[:, KC - 2:KC - 1], op0=mult, op1=add)
                b2ap = cc[:, KC - 1:KC].to_broadcast((NP, E))
                rot = ["cbA", "cbB", "cbi"]
                for i, k in enumerate(range(KC - 3, 0, -1)):
                    t_ = clen.tile([NP, E], F32, tag=f"cbt{k}")
                    nc.vector.tensor_tensor(out=t_[:], in0=w2[:], in1=b1[:], op=mult)
                    bn = cbp.tile([NP, E], F32, tag=rot[i % 3])
                    nc.vector.scalar_tensor_tensor(out=bn[:], in0=t_[:],
                                                   scalar=cc[:, k:k + 1], in1=b2ap,
                                                   op0=add, op1=sub)
                    b2ap = b1[:]
                    b1 = bn
                t_ = clen.tile([NP, E], F32, tag="cbt0")
                nc.vector.tensor_tensor(out=t_[:], in0=v[:], in1=b1[:], op=mult)
                tot = small.tile([NP, E], F32, tag="tot")
                nc.vector.scalar_tensor_tensor(out=tot[:], in0=t_[:],
                                               scalar=cc[:, 0:1], in1=b2ap,
                                               op0=add, op1=sub)
                tot_p = pchunk[:, 0:E]
                nc.scalar.activation(tot_p, tot[:], Cp)
                d.update(b15=b15, invb=invb, rawrec=rawrec, tot=tot,
                         pchunk=pchunk)

            # ---- phase 2: accept/reject + tail for both chunks ----
            for c in range(NCHUNK):
                sl = slice(c * NP, (c + 1) * NP)
                d = ch[c]
                u_t, b15, invb = d["u_t"], d["b15"], d["invb"]
                rawrec, tot, pchunk = d["rawrec"], d["tot"], d["pchunk"]
                tot_p = pchunk[:, 0:E]
                rawrec_p = pchunk[:, E:2 * E]
                rr_bd = rawrec_p.unsqueeze(1).to_broadcast((NP, DS, E))
                rr_bg = rawrec[:].unsqueeze(1).to_broadcast((NP, gs, E))

                h1 = gs // 2
                h2 = gs - h1
                # accept mask in three SEPARATE tiles (distinct tiles per
                # writer/reader pair -- slice-sharing raced on real HW) so
                # GpSimd starts multiplying after only h1 rows are compared.
                acc_g1 = slab.tile([NP, h1, E], F32, tag="accg1")
                nc.vector.scalar_tensor_tensor(out=acc_g1[:], in0=u_t[:, 0:h1, :],
                                               scalar=b15[:],
                                               in1=tot_p.unsqueeze(1).to_broadcast((NP, h1, E)),
                                               op0=mult, op1=is_lt)
                acc_g2 = slab.tile([NP, h2, E], F32, tag="accg2")
                nc.vector.scalar_tensor_tensor(out=acc_g2[:], in0=u_t[:, h1:gs, :],
                                               scalar=b15[:],
                                               in1=tot_p.unsqueeze(1).to_broadcast((NP, h2, E)),
                                               op0=mult, op1=is_lt)
                if DS > 0:
                    acc_d = slab.tile([NP, DS, E], F32, tag="accd")
                    nc.vector.scalar_tensor_tensor(out=acc_d[:], in0=u_t[:, gs:S, :],
                                                   scalar=b15[:],
                                                   in1=tot_p.unsqueeze(1).to_broadcast((NP, DS, E)),
                                                   op0=mult, op1=is_lt)
                sel_g1 = slab.tile([NP, h1, E], F32, tag="selg1")
                nc.gpsimd.tensor_tensor(out=sel_g1[:], in0=acc_g1[:],
                                        in1=rawrec[:].unsqueeze(1).to_broadcast((NP, h1, E)),
                                        op=mult)
                sel_g2 = slab.tile([NP, h2, E], F32, tag="selg2")
                nc.gpsimd.tensor_tensor(out=sel_g2[:], in0=acc_g2[:],
                                        in1=rawrec[:].unsqueeze(1).to_broadcast((NP, h2, E)),
                                        op=mult)
                if DS > 0:
                    sel_d = slab.tile([NP, DS, E], F32, tag="seld")
                    nc.vector.tensor_tensor(out=sel_d[:], in0=acc_d[:],
                                            in1=rr_bd, op=mult)
                    red_d = small.tile([NP, DS], F32, tag="redd")
                    nc.vector.reduce_max(out=red_d[:], in_=sel_d[:], axis=mybir.AxisListType.X)
                red_g1 = small.tile([NP, h1], F32, tag="redg1")
                nc.vector.reduce_max(out=red_g1[:], in_=sel_g1[:], axis=mybir.AxisListType.X)
                red_g2 = small.tile([NP, h2], F32, tag="redg2")
                nc.vector.reduce_max(out=red_g2[:], in_=sel_g2[:], axis=mybir.AxisListType.X)

                red = small.tile([NP, S], F32, tag="red")
                nc.scalar.activation(red[:, 0:h1], red_g1[:], Cp)
                nc.scalar.activation(red[:, h1:gs], red_g2[:], Cp)
                if DS > 0:
                    nc.scalar.activation(red[:, gs:S], red_d[:], Cp)

                accm = small.tile([NP, S], F32, tag="accm")
                nc.vector.reciprocal(accm[:], red[:])
                acc = small.tile([NP, S], F32, tag="acc")
                nc.scalar.activation(acc[:], accm[:], Cp, scale=invb[:])
                who = small.tile([NP, S], mybir.dt.int32, tag="who")
                nc.vector.tensor_scalar(out=who[:], in0=red[:], scalar1=0.0,
                                        scalar2=None, op0=is_gt)
                lastx = small.tile([NP, 1], F32, tag="lastx")
                nc.scalar.activation(lastx[:], d["raw_t"][:, E - 1:E], Cp, scale=invb[:])
                fb = small.tile([NP, 1], F32, tag="fb")
                nc.vector.tensor_tensor(out=fb[:], in0=lastx[:], in1=d["dtb_t"][:],
                                        op=amax)
                res_t = small.tile([NP, S], F32, tag="res")
                nc.scalar.activation(res_t[:], fb[:].to_broadcast((NP, S)), Cp)
                nc.vector.copy_predicated(res_t[:], who[:], acc[:])
                res2_t = small.tile([NP, S], F32, tag="res2")
                nc.vector.tensor_scalar(out=res2_t[:], in0=res_t[:], scalar1=1e5,
                                        scalar2=None, op0=amin)
                nc.sync.dma_start(out=res_d.ap()[sl], in_=res2_t[:])

    nc.finalize()
    return nc


def _prep_inputs(time_seq, time_delta_seq, event_seq, dtime_boundary, exp_raw,
                 unif_numbers, mu, alpha, beta, type_emb):
    f = np.float32
    tds = np.ascontiguousarray(np.asarray(time_delta_seq, f))
    dtb = np.ascontiguousarray(np.asarray(dtime_boundary, f))
    raw = np.ascontiguousarray(np.asarray(exp_raw, f))
    u = np.ascontiguousarray(np.asarray(unif_numbers, f))
    ev = np.asarray(event_seq)
    mu = np.asarray(mu, f)
    alpha = np.asarray(alpha, f)
    beta = np.asarray(beta, f)
    type_emb = np.asarray(type_emb, f)

    aemb_full = (alpha[None, :] * type_emb)[ev]            # [B,L,M]
    negbeta_bc = np.tile(-beta[None, :], (NP, 1)).astype(f)
    mu_bc = np.tile(mu[None, :], (NP, 1)).astype(f)
    tlin = np.linspace(0.0, 1.0, S0, dtype=f)
    tlin_bc = np.tile(tlin[None, :], (NP, 1)).astype(f)

    # Interpolation domain D per pair (float64 host estimate; only needs to
    # satisfy D >= xmax, which holds because bounds >= 1.5*tot(dt=0)).
    tot00 = np.log1p(np.exp((aemb_full + mu[None, None, :]).astype(np.float64))).sum(-1)
    rawmax = raw.max(-1).astype(np.float64)
    Ddom = rawmax / (1.5 * tot00)                          # [B,L]
    n = KC - 1
    jj = np.arange(KC)
    frac = (1.0 + np.cos(np.pi * jj / n)) / 2.0
    nodes_full = (Ddom[..., None] * frac[None, None, :]).astype(f)   # [B,L,KC]
    fourd_full = (4.0 / Ddom).astype(f)                    # [B,L]

    Wm = np.zeros((KC, KC))
    for k in range(KC):
        wrow = np.cos(np.pi * jj * k / n)
        wrow[0] *= 0.5
        wrow[-1] *= 0.5
        wrow *= 2.0 / n
        if k == 0 or k == n:
            wrow *= 0.5
        Wm[k] = wrow
    wfull_bc = np.tile(Wm.reshape(1, KC * KC).astype(f), (NP, 1))

    in_maps = []
    for c in range(N_CORES):
        bs = slice(c * BPC, (c + 1) * BPC)
        in_maps.append(dict(
            u=np.ascontiguousarray(u[bs].reshape(P, S, E)),
            raw=np.ascontiguousarray(raw[bs].reshape(P, E)),
            tds=np.ascontiguousarray(tds[bs].reshape(P, 1)),
            dtb=np.ascontiguousarray(dtb[bs].reshape(P, 1)),
            aemb=np.ascontiguousarray(aemb_full[bs].reshape(P, M)),
            nodes=np.ascontiguousarray(nodes_full[bs].reshape(P, KC)),
            fourd=np.ascontiguousarray(fourd_full[bs].reshape(P, 1)),
            negbeta=negbeta_bc,
            mu=mu_bc,
            tlin=tlin_bc,
            wfull=wfull_bc,
        ))
    return in_maps


def kernel(time_seq, time_delta_seq, event_seq, dtime_boundary, exp_raw,
           unif_numbers, mu, alpha, beta, type_emb, _trace=False):
    if "nc" not in _CACHE:
        _CACHE["nc"] = build_program()
    nc = _CACHE["nc"]

    in_maps = _prep_inputs(time_seq, time_delta_seq, event_seq, dtime_boundary,
                           exp_raw, unif_numbers, mu, alpha, beta, type_emb)

    out = run_bass_kernel_spmd(nc, in_maps, core_ids=list(range(N_CORES)),
                               trace=_trace)
    _CACHE["last_results"] = out

    res = np.concatenate([out.results[c]["res"].reshape(BPC, L, S)
                          for c in range(N_CORES)], axis=0)
    weights = np.full((B, L, S), np.float32(1.0 / S), np.float32)
    return res, weights

